# revision 1
# baseline (speedup 1.0000x reference)
"""Trainium2 Bass kernel for nn_ColorRenderer (SoftRas-style color renderer).

Algorithm (per pixel p, over faces f):
  winner(p) = argmax_f score(f,p),   score = min(BIG*w0, BIG*w1, BIG*w2, -depth)
  valid(p)  = maxscore > THRESH;     color(p) = winner's affine color eval.

Each of the 4 "banks" (BIG*w0, BIG*w1, BIG*w2, -depth) is an affine function
of pixel coords, evaluated on the TensorEngine as a K=9 bf16 matmul
(3-way bf16 split of each f64 coefficient; pixel coords recentered so they
are exact small integers in bf16).  The 4 banks are packed into 4 PE
row-groups (tile_position row tiling) and run concurrently.

Sharding: the host computes the global screen bbox of all projected faces
(everything outside is provably background) and deals bbox pixels
round-robin to the 8 cores.  Each core rasterizes all (padded) 1024 faces
for its pixels.  Host does only projection / coefficient prep (O(V+F)) and
final scatter of per-core pixel colors into the [1,3,256,256] frame.
"""

import numpy as np
import ml_dtypes

IMAGE_SIZE = 256
ORIG_SIZE = 512
DENOM_EPS = 1e-8

BIG = 1e14
THRESH = -5000.0
BAD = -3.0e30
NCORES = 8
FPAD = 1024
BLK = 512

bf16 = ml_dtypes.bfloat16

_PROGRAM_CACHE = {}


# ----------------------------------------------------------------------------
# Host-side math (projection, coefficients)
# ----------------------------------------------------------------------------

def _project_f32(vertices, K, R, t, dist_coeffs):
    """Faithful float32 replication of the reference projection."""
    f32 = np.float32
    EPS = f32(1e-9)
    v = np.einsum('bij,bvj->bvi', R.astype(f32), vertices.astype(f32)).astype(f32) + t.astype(f32)
    x, y, z = v[..., 0], v[..., 1], v[..., 2]
    x_ = (x / (z + EPS)).astype(f32)
    y_ = (y / (z + EPS)).astype(f32)
    r2 = (x_ * x_ + y_ * y_).astype(f32)
    d = dist_coeffs.astype(f32)
    k1 = d[:, 0:1]; k2 = d[:, 1:2]; p1 = d[:, 2:3]; p2 = d[:, 3:4]; k3 = d[:, 4:5]
    radial = (f32(1.0) + k1 * r2 + k2 * r2 ** 2 + k3 * r2 ** 3).astype(f32)
    x__ = (x_ * radial + f32(2.0) * p1 * x_ * y_ + p2 * (r2 + f32(2.0) * x_ * x_)).astype(f32)
    y__ = (y_ * radial + p1 * (r2 + f32(2.0) * y_ * y_) + f32(2.0) * p2 * x_ * y_).astype(f32)
    ones = np.ones_like(x__)
    uv = np.einsum('bij,bvj->bvi', K.astype(f32),
                   np.stack([x__, y__, ones], -1).astype(f32)).astype(f32)
    OS = f32(ORIG_SIZE)
    u = (f32(2.0) * (uv[..., 0] - OS / 2) / OS).astype(f32)
    vv = (f32(2.0) * ((OS - uv[..., 1]) - OS / 2) / OS).astype(f32)
    return np.stack([u, vv, z], -1).astype(f32)          # [B,V,3]


def _face_vertices_f32(verts, faces):
    f32 = np.float32
    IM = f32(IMAGE_SIZE)
    fv = verts[0][faces[0]]                               # [F,3,3]
    fv = fv * np.array([1.0, -1.0, 1.0], dtype=f32)
    fv = (fv * (IM / 2) + IM / 2).astype(f32)
    return fv


def _build_coeffs(fv):
    """Per-face f64 affine coefficients for w0,w1,w2,negdepth + ok mask."""
    f = fv.astype(np.float64)
    x0, y0, z0 = f[:, 0, 0], f[:, 0, 1], f[:, 0, 2]
    x1, y1, z1 = f[:, 1, 0], f[:, 1, 1], f[:, 1, 2]
    x2, y2, z2 = f[:, 2, 0], f[:, 2, 1], f[:, 2, 2]
    denom_f32 = ((fv[:, 1, 1] - fv[:, 2, 1]) * (fv[:, 0, 0] - fv[:, 2, 0])
                 + (fv[:, 2, 0] - fv[:, 1, 0]) * (fv[:, 0, 1] - fv[:, 2, 1])).astype(np.float32)
    ok = np.abs(denom_f32) > np.float32(DENOM_EPS)
    d = np.where(ok, (y1 - y2) * (x0 - x2) + (x2 - x1) * (y0 - y2), 1.0)
    a0 = (y1 - y2) / d; b0 = (x2 - x1) / d
    c0 = (-(y1 - y2) * x2 - (x2 - x1) * y2) / d
    a1 = (y2 - y0) / d; b1 = (x0 - x2) / d
    c1 = (-(y2 - y0) * x2 - (x0 - x2) * y2) / d
    a2 = (y0 - y1) / d; b2 = (x1 - x0) / d
    c2 = (-(y0 - y1) * x1 - (x1 - x0) * y1) / d
    and_ = -(a0 * z0 + a1 * z1 + a2 * z2)
    bnd = -(b0 * z0 + b1 * z1 + b2 * z2)
    cnd = -(c0 * z0 + c1 * z1 + c2 * z2)
    return dict(ok=ok, w0=(a0, b0, c0), w1=(a1, b1, c1), w2=(a2, b2, c2),
                nd=(and_, bnd, cnd))


def _split3_bf16(a):
    h = a.astype(bf16)
    r1 = a - h.astype(np.float64)
    m = r1.astype(bf16)
    l = (r1 - m.astype(np.float64)).astype(bf16)
    return h, m, l


def _bank_rows(a, b, c, sx, sy, mask_bad, bad_c):
    """9 bf16 coefficient rows for one bank (recentered at sx, sy)."""
    cc = c + a * sx + b * sy
    a = np.where(mask_bad, 0.0, a)
    b = np.where(mask_bad, 0.0, b)
    cc = np.where(mask_bad, bad_c, cc)
    ah, am, al = _split3_bf16(a)
    bh, bm, bl = _split3_bf16(b)
    ch, cm, cl = _split3_bf16(cc)
    return np.stack([ah, am, al, bh, bm, bl, ch, cm, cl], 0)   # [9, F] bf16


# ----------------------------------------------------------------------------
# Bass program
# ----------------------------------------------------------------------------

def _build_program(nch, fpad):
    STAGE = 4  # full pipeline (lower values were build-time debug bisection)
    import concourse.bacc as bacc
    import concourse.tile as tile
    import concourse.bass as bass
    from concourse import mybir
    from contextlib import ExitStack

    dt = mybir.dt
    op = mybir.AluOpType
    nc = bacc.Bacc("TRN2", target_bir_lowering=False, debug=False,
                   num_devices=NCORES)

    pixlhs = nc.dram_tensor("pixlhs", [128, nch * 128], dt.bfloat16, kind="ExternalInput")
    coefs = nc.dram_tensor("coefs", [128, fpad], dt.bfloat16, kind="ExternalInput")
    pxf = nc.dram_tensor("pxf", [128, nch], dt.float32, kind="ExternalInput")
    pyf = nc.dram_tensor("pyf", [128, nch], dt.float32, kind="ExternalInput")
    colco = nc.dram_tensor("colco", [fpad, 16], dt.float32, kind="ExternalInput")
    colout = nc.dram_tensor("colout", [3, 128, nch], dt.float32, kind="ExternalOutput")
    dbgmax = nc.dram_tensor("dbgmax", [128, nch], dt.float32, kind="ExternalOutput")
    dbgidx = nc.dram_tensor("dbgidx", [128, nch], dt.uint32, kind="ExternalOutput")

    nblk = fpad // BLK

    with tile.TileContext(nc) as tc, ExitStack() as ctx:
        const = ctx.enter_context(tc.tile_pool(name="const", bufs=1))
        psum = ctx.enter_context(tc.tile_pool(name="psum", bufs=2, space="PSUM"))
        work = ctx.enter_context(tc.tile_pool(name="work", bufs=4))
        scorep = ctx.enter_context(tc.tile_pool(name="scorep", bufs=2))
        accp = ctx.enter_context(tc.tile_pool(name="accp", bufs=1))

        pix_sb = const.tile([128, nch * 128], dt.bfloat16)
        nc.sync.dma_start(out=pix_sb[:], in_=pixlhs[:])
        coef_sb = const.tile([128, fpad], dt.bfloat16)
        nc.sync.dma_start(out=coef_sb[:], in_=coefs[:])
        pxf_sb = const.tile([128, nch], dt.float32)
        nc.sync.dma_start(out=pxf_sb[:], in_=pxf[:])
        pyf_sb = const.tile([128, nch], dt.float32)
        nc.sync.dma_start(out=pyf_sb[:], in_=pyf[:])

        maxall = accp.tile([128, nch], dt.float32)
        idx32 = accp.tile([128, nch], dt.uint32)
        gath = accp.tile([128, nch, 16], dt.float32)

        for j in range(nch):
            score = scorep.tile([128, fpad], dt.float32, tag="score")
            for b in range(nblk):
                banks = []
                for g in range(4):
                    pt = psum.tile([128, BLK], dt.float32, tag=f"bank{g}")
                    nc.tensor.matmul(
                        pt[:],
                        pix_sb[32 * g:32 * g + 9, j * 128:(j + 1) * 128],
                        coef_sb[32 * g:32 * g + 9, b * BLK:(b + 1) * BLK],
                        start=True, stop=True,
                        tile_position=(32 * g, 0),
                    )
                    banks.append(pt)
                # TT can read only one PSUM operand: ACT copies banks 0,2 to SBUF
                a_sb = work.tile([128, BLK], dt.float32, tag="a_sb")
                c_sb = work.tile([128, BLK], dt.float32, tag="c_sb")
                nc.scalar.copy(out=a_sb[:], in_=banks[0][:])
                nc.scalar.copy(out=c_sb[:], in_=banks[2][:])
                x = work.tile([128, BLK], dt.float32, tag="x")
                y = work.tile([128, BLK], dt.float32, tag="y")
                nc.vector.tensor_tensor(out=x[:], in0=a_sb[:], in1=banks[1][:], op=op.min)
                nc.vector.tensor_tensor(out=y[:], in0=c_sb[:], in1=banks[3][:], op=op.min)
                nc.vector.tensor_tensor(
                    out=score[:, b * BLK:(b + 1) * BLK],
                    in0=x[:], in1=y[:], op=op.min)
            cmx = work.tile([128, 1], dt.float32, tag="cmx")
            nc.vector.tensor_reduce(out=cmx[:], in_=score[:], axis=mybir.AxisListType.X, op=op.max)
            nc.scalar.copy(out=maxall[:, j:j + 1], in_=cmx[:])
            if STAGE >= 2:
                # broadcast chunk max to the 8-wide in_max format of max_index
                cm_ap = cmx[:, 0:1]
                cm8 = bass.AP(tensor=cm_ap.tensor, offset=cm_ap.offset,
                              ap=[cm_ap.ap[0], [0, 8]])
                idx8 = work.tile([128, 8], dt.uint32, tag="idx8")
                nc.vector.max_index(out=idx8[:], in_max=cm8, in_values=score[:])
                nc.vector.tensor_copy(out=idx32[:, j:j + 1], in_=idx8[:, 0:1])

        if STAGE < 2:
            nc.vector.memset(idx32[:], 0)
        # phase 2: gather per-pixel winner color coefficients, eval affine
        if STAGE >= 3:
            for j in range(nch):
                nc.gpsimd.indirect_dma_start(
                    out=gath[:, j, :], out_offset=None,
                    in_=colco[:],
                    in_offset=bass.IndirectOffsetOnAxis(ap=idx32[:, j:j + 1], axis=0),
                    bounds_check=fpad - 1, oob_is_err=False,
                )
        else:
            nc.vector.memset(gath[:], 0)
        vmask = work.tile([128, nch], dt.float32, tag="vmask")
        nc.vector.tensor_scalar(out=vmask[:], in0=maxall[:], scalar1=float(THRESH),
                                scalar2=None, op0=op.is_gt)
        nc.sync.dma_start(out=dbgmax[:], in_=maxall[:])
        nc.sync.dma_start(out=dbgidx[:], in_=idx32[:])
        for ch in range(3):
            m1 = work.tile([128, nch], dt.float32, tag="m1")
            m2 = work.tile([128, nch], dt.float32, tag="m2")
            cv = work.tile([128, nch], dt.float32, tag="cv")
            nc.vector.tensor_tensor(out=m1[:], in0=gath[:, :, 3 * ch + 0], in1=pxf_sb[:], op=op.mult)
            nc.vector.tensor_tensor(out=m2[:], in0=gath[:, :, 3 * ch + 1], in1=pyf_sb[:], op=op.mult)
            nc.vector.tensor_tensor(out=m1[:], in0=m1[:], in1=m2[:], op=op.add)
            nc.vector.tensor_tensor(out=m1[:], in0=m1[:], in1=gath[:, :, 3 * ch + 2], op=op.add)
            nc.vector.tensor_tensor(out=cv[:], in0=m1[:], in1=vmask[:], op=op.mult)
            nc.sync.dma_start(out=colout[ch], in_=cv[:])

    nc.compile()
    return nc


def _get_program(nch, fpad):
    key = (nch, fpad)
    if key not in _PROGRAM_CACHE:
        _PROGRAM_CACHE[key] = _build_program(nch, fpad)
    return _PROGRAM_CACHE[key]


# ----------------------------------------------------------------------------
# Host orchestration
# ----------------------------------------------------------------------------

def prepare(vertices, faces, textures, K, R, t, dist_coeffs):
    """All host-side prep.  Returns (nch, in_maps, scatter_info)."""
    verts = _project_f32(np.asarray(vertices), np.asarray(K), np.asarray(R),
                         np.asarray(t), np.asarray(dist_coeffs))
    fv = _face_vertices_f32(verts, np.asarray(faces))
    F = fv.shape[0]
    co = _build_coeffs(fv)
    bad = ~co['ok']
    tex = np.asarray(textures)[0].astype(np.float64)      # [F,3,C]

    # global bbox (+1px margin) of all face vertices
    xmin, xmax = fv[:, :, 0].min(), fv[:, :, 0].max()
    ymin, ymax = fv[:, :, 1].min(), fv[:, :, 1].max()
    c_lo = max(0, int(np.floor(xmin - 0.5)) - 1)
    c_hi = min(IMAGE_SIZE - 1, int(np.ceil(xmax - 0.5)) + 1)
    r_lo = max(0, int(np.floor(ymin - 0.5)) - 1)
    r_hi = min(IMAGE_SIZE - 1, int(np.ceil(ymax - 0.5)) + 1)
    if c_hi < c_lo or r_hi < r_lo:
        return None                                        # nothing visible

    ncols = c_hi - c_lo + 1
    nrows = r_hi - r_lo + 1
    G = nrows * ncols
    nch = max(1, -(-G // (NCORES * 128)))
    S = nch * 128

    # centered pixel coords must be bf16-exact small integers
    sx = np.floor((c_lo + c_hi) / 2) + 0.5
    sy = np.floor((r_lo + r_hi) / 2) + 0.5
    span = max(c_hi - c_lo, r_hi - r_lo) / 2 + 2
    assert span <= 192, "bbox too large for bf16-exact centered coords"

    fpad = max(BLK, -(-F // BLK) * BLK)
    pad = np.zeros(fpad - F, dtype=bool)
    badp = np.concatenate([bad, ~pad])                     # padded faces are bad

    def padded(a):
        return np.concatenate([a, np.zeros(fpad - F, dtype=np.float64)])

    # score banks (w scaled by BIG, negdepth unscaled)
    coefs = np.zeros((128, fpad), dtype=bf16)
    for g, name in enumerate(['w0', 'w1', 'w2']):
        a, b, c = (padded(v) * BIG for v in co[name])
        coefs[32 * g:32 * g + 9, :] = _bank_rows(a, b, c, sx, sy, badp, BAD)
    a, b, c = (padded(v) for v in co['nd'])
    coefs[96:96 + 9, :] = _bank_rows(a, b, c, sx, sy, badp, BAD)

    # color affine coefficients table [fpad, 16] f32 (recentered)
    colco = np.zeros((fpad, 16), dtype=np.float32)
    for ch in range(3):
        t0, t1, t2 = tex[:, 0, ch], tex[:, 1, ch], tex[:, 2, ch]
        A = padded(co['w0'][0] * t0 + co['w1'][0] * t1 + co['w2'][0] * t2)
        B = padded(co['w0'][1] * t0 + co['w1'][1] * t1 + co['w2'][1] * t2)
        C = padded(co['w0'][2] * t0 + co['w1'][2] * t1 + co['w2'][2] * t2)
        C = C + A * sx + B * sy
        colco[:, 3 * ch + 0] = A.astype(np.float32)
        colco[:, 3 * ch + 1] = B.astype(np.float32)
        colco[:, 3 * ch + 2] = C.astype(np.float32)

    # pixel lists per core: global bbox pixel g -> core g%8, slot g//8
    # slot s <-> (partition p, chunk j): s = p*nch + j
    in_maps = []
    rows_of = np.empty((NCORES, S), dtype=np.int32)
    cols_of = np.empty((NCORES, S), dtype=np.int32)
    real_of = np.empty((NCORES, S), dtype=bool)
    for k in range(NCORES):
        g = np.arange(S) * NCORES + k
        real = g < G
        gc = np.where(real, g, 0)
        rr = gc // ncols + r_lo
        cc = gc % ncols + c_lo
        rows_of[k] = rr; cols_of[k] = cc; real_of[k] = real
        pxc = (cc + 0.5) - sx                              # exact ints
        pyc = (rr + 0.5) - sy
        # s = p*nch + j ; lhsT column index = j*128 + p
        s = np.arange(S)
        p = s // nch
        j = s % nch
        colidx = j * 128 + p
        pixlhs = np.zeros((128, S), dtype=bf16)
        pxb = pxc.astype(bf16); pyb = pyc.astype(bf16)
        assert np.all(pxb.astype(np.float64) == pxc)
        assert np.all(pyb.astype(np.float64) == pyc)
        onerow = np.ones(S, dtype=bf16)
        rows9 = [pxb, pxb, pxb, pyb, pyb, pyb, onerow, onerow, onerow]
        for gg in range(4):
            for r in range(9):
                pixlhs[32 * gg + r, colidx] = rows9[r]
        pxf = np.zeros((128, nch), dtype=np.float32)
        pyf = np.zeros((128, nch), dtype=np.float32)
        pxf[p, j] = pxc.astype(np.float32)
        pyf[p, j] = pyc.astype(np.float32)
        in_maps.append(dict(pixlhs=pixlhs, coefs=coefs, pxf=pxf, pyf=pyf,
                            colco=colco))

    scatter = dict(rows_of=rows_of, cols_of=cols_of, real_of=real_of, nch=nch,
                   fpad=fpad)
    return nch, fpad, in_maps, scatter


def assemble(results, scatter):
    out = np.zeros((1, 3, IMAGE_SIZE, IMAGE_SIZE), dtype=np.float32)
    nch = scatter['nch']
    for k in range(NCORES):
        col = results[k]['colout']                         # [3, 128, nch]
        flat = col.reshape(3, 128 * nch)                   # slot s = p*nch+j
        real = scatter['real_of'][k]
        rr = scatter['rows_of'][k][real]
        cc = scatter['cols_of'][k][real]
        out[0, :, rr, cc] = flat[:, real].T
    return out


def kernel(**inputs):
    from concourse.bass_utils import run_bass_kernel_spmd

    prep = prepare(**inputs)
    if prep is None:
        return np.zeros((1, 3, IMAGE_SIZE, IMAGE_SIZE), dtype=np.float32)
    nch, fpad, in_maps, scatter = prep
    nc = _get_program(nch, fpad)
    res = run_bass_kernel_spmd(nc, in_maps, core_ids=list(range(NCORES)))
    return assemble(res.results, scatter)


if __name__ == "__main__":
    pass



# revision 12
# speedup vs baseline: 1.1134x; 1.1134x over previous
"""Trainium2 Bass kernel for nn_ColorRenderer (SoftRas-style color renderer).

Algorithm (per pixel p, over faces f):
  nS(p,f) = max(B0, B1, B2, B3) where B_g = d - BIG*w_g (g=0..2), B_3 = d.
  For covered pixels all w_g >= 0 so nS = d (the face depth, bit-exact from
  the B3 matmul); for non-covered pixels nS ~ BIG*|w| >> 5000.
  winner(p) = argmin_f nS;  valid(p) = min_f nS < 5000.

Each bank is an affine function of pixel coords, evaluated on the
TensorEngine as a K=9 bf16 matmul (3-way bf16 split of each f64
coefficient; pixel coords recentered so they are exact small integers in
bf16).  The 4 banks occupy 4 PE row-groups (tile_position row tiling).

Per face-block the 4 PSUM banks are combined with one ACT copy + two
tensor maxes (split over Pool/DVE) + one DVE tensor_tensor_reduce that
also chains the per-chunk running min.  The argmin index is extracted
with a single Pool scalar_tensor_tensor: sum((nS <= M) * iota).

Sharding: the host computes the global screen bbox of all projected faces
(everything outside is provably background) and deals bbox pixels
round-robin to the 8 cores.  Each core rasterizes all (padded) faces for
its pixels.  Host does projection / coefficient prep (O(V+F)) and the
final per-pixel affine color eval + scatter (O(P)).
"""

import numpy as np
import ml_dtypes

IMAGE_SIZE = 256
ORIG_SIZE = 512
DENOM_EPS = 1e-8

BIG = 1e14
THRESH_POS = 5000.0
BADPOS = 3.0e30
NCORES = 8
BLK = 512

bf16 = ml_dtypes.bfloat16

_PROGRAM_CACHE = {}


# ----------------------------------------------------------------------------
# Host-side math (projection, coefficients)
# ----------------------------------------------------------------------------

def _project_f32(vertices, K, R, t, dist_coeffs):
    """Faithful float32 replication of the reference projection."""
    f32 = np.float32
    EPS = f32(1e-9)
    v = np.einsum('bij,bvj->bvi', R.astype(f32), vertices.astype(f32)).astype(f32) + t.astype(f32)
    x, y, z = v[..., 0], v[..., 1], v[..., 2]
    x_ = (x / (z + EPS)).astype(f32)
    y_ = (y / (z + EPS)).astype(f32)
    r2 = (x_ * x_ + y_ * y_).astype(f32)
    d = dist_coeffs.astype(f32)
    k1 = d[:, 0:1]; k2 = d[:, 1:2]; p1 = d[:, 2:3]; p2 = d[:, 3:4]; k3 = d[:, 4:5]
    radial = (f32(1.0) + k1 * r2 + k2 * r2 ** 2 + k3 * r2 ** 3).astype(f32)
    x__ = (x_ * radial + f32(2.0) * p1 * x_ * y_ + p2 * (r2 + f32(2.0) * x_ * x_)).astype(f32)
    y__ = (y_ * radial + p1 * (r2 + f32(2.0) * y_ * y_) + f32(2.0) * p2 * x_ * y_).astype(f32)
    ones = np.ones_like(x__)
    uv = np.einsum('bij,bvj->bvi', K.astype(f32),
                   np.stack([x__, y__, ones], -1).astype(f32)).astype(f32)
    OS = f32(ORIG_SIZE)
    u = (f32(2.0) * (uv[..., 0] - OS / 2) / OS).astype(f32)
    vv = (f32(2.0) * ((OS - uv[..., 1]) - OS / 2) / OS).astype(f32)
    return np.stack([u, vv, z], -1).astype(f32)          # [B,V,3]


def _face_vertices_f32(verts, faces):
    f32 = np.float32
    IM = f32(IMAGE_SIZE)
    fv = verts[0][faces[0]]                               # [F,3,3]
    fv = fv * np.array([1.0, -1.0, 1.0], dtype=f32)
    fv = (fv * (IM / 2) + IM / 2).astype(f32)
    return fv


def _build_coeffs(fv):
    """Per-face f64 affine coefficients for w0,w1,w2,depth + ok mask."""
    f = fv.astype(np.float64)
    x0, y0, z0 = f[:, 0, 0], f[:, 0, 1], f[:, 0, 2]
    x1, y1, z1 = f[:, 1, 0], f[:, 1, 1], f[:, 1, 2]
    x2, y2, z2 = f[:, 2, 0], f[:, 2, 1], f[:, 2, 2]
    denom_f32 = ((fv[:, 1, 1] - fv[:, 2, 1]) * (fv[:, 0, 0] - fv[:, 2, 0])
                 + (fv[:, 2, 0] - fv[:, 1, 0]) * (fv[:, 0, 1] - fv[:, 2, 1])).astype(np.float32)
    ok = np.abs(denom_f32) > np.float32(DENOM_EPS)
    d = np.where(ok, (y1 - y2) * (x0 - x2) + (x2 - x1) * (y0 - y2), 1.0)
    a0 = (y1 - y2) / d; b0 = (x2 - x1) / d
    c0 = (-(y1 - y2) * x2 - (x2 - x1) * y2) / d
    a1 = (y2 - y0) / d; b1 = (x0 - x2) / d
    c1 = (-(y2 - y0) * x2 - (x0 - x2) * y2) / d
    a2 = (y0 - y1) / d; b2 = (x1 - x0) / d
    c2 = (-(y0 - y1) * x1 - (x1 - x0) * y1) / d
    ad = a0 * z0 + a1 * z1 + a2 * z2
    bd = b0 * z0 + b1 * z1 + b2 * z2
    cd = c0 * z0 + c1 * z1 + c2 * z2
    return dict(ok=ok, w0=(a0, b0, c0), w1=(a1, b1, c1), w2=(a2, b2, c2),
                dep=(ad, bd, cd))


def _split3_bf16(a):
    h = a.astype(bf16)
    r1 = a - h.astype(np.float64)
    m = r1.astype(bf16)
    l = (r1 - m.astype(np.float64)).astype(bf16)
    return h, m, l


def _bank_rows(a, b, c, sx, sy, mask_bad):
    """9 bf16 coefficient rows for one bank (recentered at sx, sy)."""
    cc = c + a * sx + b * sy
    a = np.where(mask_bad, 0.0, a)
    b = np.where(mask_bad, 0.0, b)
    cc = np.where(mask_bad, BADPOS, cc)
    ah, am, al = _split3_bf16(a)
    bh, bm, bl = _split3_bf16(b)
    ch, cm, cl = _split3_bf16(cc)
    return np.stack([ah, am, al, bh, bm, bl, ch, cm, cl], 0)   # [9, F] bf16


# ----------------------------------------------------------------------------
# Bass program
# ----------------------------------------------------------------------------

def _build_program(nch, fpad):
    import concourse.bacc as bacc
    import concourse.tile as tile
    import concourse.bass as bass
    from concourse import mybir
    from contextlib import ExitStack

    dt = mybir.dt
    op = mybir.AluOpType
    nc = bacc.Bacc("TRN2", target_bir_lowering=False, debug=False,
                   num_devices=NCORES)

    pixlhs = nc.dram_tensor("pixlhs", [128, nch * 128], dt.bfloat16, kind="ExternalInput")
    coefs = nc.dram_tensor("coefs", [128, fpad], dt.bfloat16, kind="ExternalInput")
    maxout = nc.dram_tensor("maxout", [128, nch], dt.float32, kind="ExternalOutput")
    idxout = nc.dram_tensor("idxout", [128, nch], dt.float32, kind="ExternalOutput")
    iotain = nc.dram_tensor("iotain", [128, fpad], dt.float32, kind="ExternalInput")

    assert fpad % BLK == 0
    nmm = fpad // BLK          # matmuls per bank (each writes a BLK-wide slice)

    with tile.TileContext(nc) as tc, ExitStack() as ctx:
        const = ctx.enter_context(tc.tile_pool(name="const", bufs=1))
        psum = ctx.enter_context(tc.tile_pool(name="psum", bufs=1, space="PSUM"))
        work = ctx.enter_context(tc.tile_pool(name="work", bufs=3))
        scorep = ctx.enter_context(tc.tile_pool(name="scorep", bufs=2))
        accp = ctx.enter_context(tc.tile_pool(name="accp", bufs=1))

        coef_sb = const.tile([128, fpad], dt.bfloat16)
        nc.sync.dma_start(out=coef_sb[:], in_=coefs[:])
        pix_sb = const.tile([128, nch * 128], dt.bfloat16)
        nc.sync.dma_start(out=pix_sb[:], in_=pixlhs[:])

        iota_f = const.tile([128, fpad], dt.float32)
        nc.sync.dma_start(out=iota_f[:], in_=iotain[:])

        maxall = accp.tile([128, nch], dt.float32)
        idxacc = accp.tile([128, nch], dt.float32)

        for j in range(nch):
            score = scorep.tile([128, fpad], dt.float32, tag="score")
            scr2 = scorep.tile([128, fpad], dt.float32, tag="scr2")
            # full-width PSUM banks (4 x fpad f32 = all 8 hw banks, bufs=1);
            # each bank is released by its (early) consumer so the next
            # chunk's matmuls chase the combine chain.
            banks = []
            for g in range(4):
                pt = psum.tile([128, fpad], dt.float32, tag=f"bank{g}")
                for b in range(nmm):
                    nc.tensor.matmul(
                        pt[:, b * BLK:(b + 1) * BLK],
                        pix_sb[32 * g:32 * g + 9, j * 128:(j + 1) * 128],
                        coef_sb[32 * g:32 * g + 9, b * BLK:(b + 1) * BLK],
                        start=True, stop=True,
                        tile_position=(32 * g, 0),
                    )
                banks.append(pt)
            # only ACT (copy) and DVE can read PSUM; Pool has no legal
            # elementwise path through this walrus pipeline.
            s0 = work.tile([128, fpad], dt.float32, tag="s0")
            nc.scalar.copy(out=s0[:], in_=banks[0][:])
            t1 = work.tile([128, fpad], dt.float32, tag="t1")
            nc.vector.tensor_tensor(out=t1[:], in0=s0[:], in1=banks[1][:], op=op.max)
            t2 = work.tile([128, fpad], dt.float32, tag="t2")
            nc.vector.tensor_tensor(out=t2[:], in0=t1[:], in1=banks[2][:], op=op.max)
            nc.vector.tensor_tensor(out=score[:], in0=t2[:], in1=banks[3][:], op=op.max)
            nc.vector.tensor_reduce(out=maxall[:, j:j + 1], in_=score[:],
                                    axis=mybir.AxisListType.X, op=op.min)
            # winner index: sum over f of (nS <= M) * f
            nc.vector.scalar_tensor_tensor(
                out=scr2[:], in0=score[:], scalar=maxall[:, j:j + 1],
                in1=iota_f[:], op0=op.is_le, op1=op.mult,
                accum_out=idxacc[:, j:j + 1],
            )

        nc.sync.dma_start(out=maxout[:], in_=maxall[:])
        nc.sync.dma_start(out=idxout[:], in_=idxacc[:])

    nc.compile()
    return nc


def _get_program(nch, fpad):
    key = (nch, fpad)
    if key not in _PROGRAM_CACHE:
        _PROGRAM_CACHE[key] = _build_program(nch, fpad)
    return _PROGRAM_CACHE[key]


# ----------------------------------------------------------------------------
# Host orchestration
# ----------------------------------------------------------------------------

def prepare(vertices, faces, textures, K, R, t, dist_coeffs):
    """All host-side prep.  Returns (nch, fpad, in_maps, scatter_info)."""
    verts = _project_f32(np.asarray(vertices), np.asarray(K), np.asarray(R),
                         np.asarray(t), np.asarray(dist_coeffs))
    fv = _face_vertices_f32(verts, np.asarray(faces))
    F = fv.shape[0]
    co = _build_coeffs(fv)
    bad = ~co['ok']
    tex = np.asarray(textures)[0].astype(np.float64)      # [F,3,C]

    # global bbox (+1px margin) of all face vertices
    xmin, xmax = fv[:, :, 0].min(), fv[:, :, 0].max()
    ymin, ymax = fv[:, :, 1].min(), fv[:, :, 1].max()
    c_lo = max(0, int(np.floor(xmin - 0.5)) - 1)
    c_hi = min(IMAGE_SIZE - 1, int(np.ceil(xmax - 0.5)) + 1)
    r_lo = max(0, int(np.floor(ymin - 0.5)) - 1)
    r_hi = min(IMAGE_SIZE - 1, int(np.ceil(ymax - 0.5)) + 1)
    if c_hi < c_lo or r_hi < r_lo:
        return None                                        # nothing visible

    ncols = c_hi - c_lo + 1
    nrows = r_hi - r_lo + 1
    G = nrows * ncols
    nch = max(1, -(-G // (NCORES * 128)))
    S = nch * 128

    # centered pixel coords must be bf16-exact small integers
    sx = np.floor((c_lo + c_hi) / 2) + 0.5
    sy = np.floor((r_lo + r_hi) / 2) + 0.5
    assert max(c_hi - c_lo, r_hi - r_lo) / 2 + 2 <= 192, "bbox too large"

    fpad = max(BLK, -(-F // BLK) * BLK)
    pad = np.zeros(fpad - F, dtype=bool)
    badp = np.concatenate([bad, ~pad])                     # padded faces are bad

    def padded(a):
        return np.concatenate([a, np.zeros(fpad - F, dtype=np.float64)])

    # banks: B_g = dep - BIG*w_g (g=0..2), B_3 = dep
    dep = tuple(padded(v) for v in co['dep'])
    coefs = np.zeros((128, fpad), dtype=bf16)
    for g, name in enumerate(['w0', 'w1', 'w2']):
        w = tuple(padded(v) for v in co[name])
        a = dep[0] - BIG * w[0]
        b = dep[1] - BIG * w[1]
        c = dep[2] - BIG * w[2]
        coefs[32 * g:32 * g + 9, :] = _bank_rows(a, b, c, sx, sy, badp)
    coefs[96:96 + 9, :] = _bank_rows(dep[0], dep[1], dep[2], sx, sy, badp)

    # host-side color affine coefficients [F, 3ch, 3] f64 (recentered)
    colco = np.zeros((fpad, 3, 3), dtype=np.float64)
    for ch in range(3):
        t0, t1, t2 = tex[:, 0, ch], tex[:, 1, ch], tex[:, 2, ch]
        A = padded(co['w0'][0] * t0 + co['w1'][0] * t1 + co['w2'][0] * t2)
        B = padded(co['w0'][1] * t0 + co['w1'][1] * t1 + co['w2'][1] * t2)
        C = padded(co['w0'][2] * t0 + co['w1'][2] * t1 + co['w2'][2] * t2)
        colco[:, ch, 0] = A
        colco[:, ch, 1] = B
        colco[:, ch, 2] = C + A * sx + B * sy

    # pixel lists per core: global bbox pixel g -> core g%8, slot g//8
    # slot s <-> (partition p, chunk j): s = p*nch + j
    iota_np = np.broadcast_to(np.arange(fpad, dtype=np.float32), (128, fpad)).copy()
    in_maps = []
    rows_of = np.empty((NCORES, S), dtype=np.int32)
    cols_of = np.empty((NCORES, S), dtype=np.int32)
    real_of = np.empty((NCORES, S), dtype=bool)
    pxc_of = np.empty((NCORES, S), dtype=np.float64)
    pyc_of = np.empty((NCORES, S), dtype=np.float64)
    for k in range(NCORES):
        g = np.arange(S) * NCORES + k
        real = g < G
        gc = np.where(real, g, 0)
        rr = gc // ncols + r_lo
        cc = gc % ncols + c_lo
        rows_of[k] = rr; cols_of[k] = cc; real_of[k] = real
        pxc = (cc + 0.5) - sx                              # exact ints
        pyc = (rr + 0.5) - sy
        pxc_of[k] = pxc; pyc_of[k] = pyc
        # s = p*nch + j ; lhsT column index within chunk j is p
        s = np.arange(S)
        p = s // nch
        j = s % nch
        colidx = j * 128 + p
        pixlhs = np.zeros((128, S), dtype=bf16)
        pxb = pxc.astype(bf16); pyb = pyc.astype(bf16)
        assert np.all(pxb.astype(np.float64) == pxc)
        assert np.all(pyb.astype(np.float64) == pyc)
        onerow = np.ones(S, dtype=bf16)
        rows9 = [pxb, pxb, pxb, pyb, pyb, pyb, onerow, onerow, onerow]
        for gg in range(4):
            for r in range(9):
                pixlhs[32 * gg + r, colidx] = rows9[r]
        in_maps.append(dict(pixlhs=pixlhs, coefs=coefs, iotain=iota_np))

    scatter = dict(rows_of=rows_of, cols_of=cols_of, real_of=real_of,
                   pxc_of=pxc_of, pyc_of=pyc_of, colco=colco,
                   nch=nch, fpad=fpad)
    return nch, fpad, in_maps, scatter


def assemble(results, scatter):
    out = np.zeros((1, 3, IMAGE_SIZE, IMAGE_SIZE), dtype=np.float32)
    nch = scatter['nch']
    fpad = scatter['fpad']
    colco = scatter['colco']
    for k in range(NCORES):
        mx = np.asarray(results[k]['maxout'], dtype=np.float64)  # [128, nch]
        ix = np.asarray(results[k]['idxout'], dtype=np.float64)
        # slot s = p*nch + j  ->  flatten [p, j] C-order gives slot order
        mxs = mx.reshape(-1)
        ixs = np.rint(ix.reshape(-1)).astype(np.int64)
        valid = (mxs < THRESH_POS) & (ixs >= 0) & (ixs < fpad)
        sel = scatter['real_of'][k] & valid
        if not np.any(sel):
            continue
        idx = ixs[sel]
        px = scatter['pxc_of'][k][sel]
        py = scatter['pyc_of'][k][sel]
        A = colco[idx]                                     # [n, 3ch, 3]
        col = A[:, :, 0] * px[:, None] + A[:, :, 1] * py[:, None] + A[:, :, 2]
        rr = scatter['rows_of'][k][sel]
        cc = scatter['cols_of'][k][sel]
        out[0, :, rr, cc] = col.astype(np.float32)
    return out


def kernel(**inputs):
    from concourse.bass_utils import run_bass_kernel_spmd

    prep = prepare(**inputs)
    if prep is None:
        return np.zeros((1, 3, IMAGE_SIZE, IMAGE_SIZE), dtype=np.float32)
    nch, fpad, in_maps, scatter = prep
    nc = _get_program(nch, fpad)
    res = run_bass_kernel_spmd(nc, in_maps, core_ids=list(range(NCORES)))
    return assemble(res.results, scatter)


if __name__ == "__main__":
    pass


# revision 13
# speedup vs baseline: 1.1837x; 1.0632x over previous
"""Trainium2 Bass kernel for nn_ColorRenderer (SoftRas-style color renderer).

Algorithm (per pixel p, over faces f):
  nS(p,f) = max(B0, B1, B2, B3) where B_g = d - BIG*w_g (g=0..2), B_3 = d.
  For covered pixels all w_g >= 0 so nS = d (the face depth, bit-exact from
  the B3 matmul); for non-covered pixels nS ~ BIG*|w| >> 5000.
  winner(p) = argmin_f nS;  valid(p) = min_f nS < 5000.

Each bank is an affine function of pixel coords, evaluated on the
TensorEngine as a K=9 bf16 matmul (3-way bf16 split of each f64
coefficient; pixel coords recentered so they are exact small integers in
bf16).  The 4 banks occupy 4 PE row-groups (tile_position row tiling).

Per face-block the 4 PSUM banks are combined with one ACT copy + two
tensor maxes (split over Pool/DVE) + one DVE tensor_tensor_reduce that
also chains the per-chunk running min.  The argmin index is extracted
with a single Pool scalar_tensor_tensor: sum((nS <= M) * iota).

Sharding: the host computes the global screen bbox of all projected faces
(everything outside is provably background) and deals bbox pixels
round-robin to the 8 cores.  Each core rasterizes all (padded) faces for
its pixels.  Host does projection / coefficient prep (O(V+F)) and the
final per-pixel affine color eval + scatter (O(P)).
"""

import numpy as np
import ml_dtypes

IMAGE_SIZE = 256
ORIG_SIZE = 512
DENOM_EPS = 1e-8

BIG = 1e14
THRESH_POS = 5000.0
BADPOS = 3.0e30
NCORES = 8
BLK = 512

bf16 = ml_dtypes.bfloat16

_PROGRAM_CACHE = {}


# ----------------------------------------------------------------------------
# Host-side math (projection, coefficients)
# ----------------------------------------------------------------------------

def _project_f32(vertices, K, R, t, dist_coeffs):
    """Faithful float32 replication of the reference projection."""
    f32 = np.float32
    EPS = f32(1e-9)
    v = np.einsum('bij,bvj->bvi', R.astype(f32), vertices.astype(f32)).astype(f32) + t.astype(f32)
    x, y, z = v[..., 0], v[..., 1], v[..., 2]
    x_ = (x / (z + EPS)).astype(f32)
    y_ = (y / (z + EPS)).astype(f32)
    r2 = (x_ * x_ + y_ * y_).astype(f32)
    d = dist_coeffs.astype(f32)
    k1 = d[:, 0:1]; k2 = d[:, 1:2]; p1 = d[:, 2:3]; p2 = d[:, 3:4]; k3 = d[:, 4:5]
    radial = (f32(1.0) + k1 * r2 + k2 * r2 ** 2 + k3 * r2 ** 3).astype(f32)
    x__ = (x_ * radial + f32(2.0) * p1 * x_ * y_ + p2 * (r2 + f32(2.0) * x_ * x_)).astype(f32)
    y__ = (y_ * radial + p1 * (r2 + f32(2.0) * y_ * y_) + f32(2.0) * p2 * x_ * y_).astype(f32)
    ones = np.ones_like(x__)
    uv = np.einsum('bij,bvj->bvi', K.astype(f32),
                   np.stack([x__, y__, ones], -1).astype(f32)).astype(f32)
    OS = f32(ORIG_SIZE)
    u = (f32(2.0) * (uv[..., 0] - OS / 2) / OS).astype(f32)
    vv = (f32(2.0) * ((OS - uv[..., 1]) - OS / 2) / OS).astype(f32)
    return np.stack([u, vv, z], -1).astype(f32)          # [B,V,3]


def _face_vertices_f32(verts, faces):
    f32 = np.float32
    IM = f32(IMAGE_SIZE)
    fv = verts[0][faces[0]]                               # [F,3,3]
    fv = fv * np.array([1.0, -1.0, 1.0], dtype=f32)
    fv = (fv * (IM / 2) + IM / 2).astype(f32)
    return fv


def _build_coeffs(fv):
    """Per-face f64 affine coefficients for w0,w1,w2,depth + ok mask."""
    f = fv.astype(np.float64)
    x0, y0, z0 = f[:, 0, 0], f[:, 0, 1], f[:, 0, 2]
    x1, y1, z1 = f[:, 1, 0], f[:, 1, 1], f[:, 1, 2]
    x2, y2, z2 = f[:, 2, 0], f[:, 2, 1], f[:, 2, 2]
    denom_f32 = ((fv[:, 1, 1] - fv[:, 2, 1]) * (fv[:, 0, 0] - fv[:, 2, 0])
                 + (fv[:, 2, 0] - fv[:, 1, 0]) * (fv[:, 0, 1] - fv[:, 2, 1])).astype(np.float32)
    ok = np.abs(denom_f32) > np.float32(DENOM_EPS)
    d = np.where(ok, (y1 - y2) * (x0 - x2) + (x2 - x1) * (y0 - y2), 1.0)
    a0 = (y1 - y2) / d; b0 = (x2 - x1) / d
    c0 = (-(y1 - y2) * x2 - (x2 - x1) * y2) / d
    a1 = (y2 - y0) / d; b1 = (x0 - x2) / d
    c1 = (-(y2 - y0) * x2 - (x0 - x2) * y2) / d
    a2 = (y0 - y1) / d; b2 = (x1 - x0) / d
    c2 = (-(y0 - y1) * x1 - (x1 - x0) * y1) / d
    ad = a0 * z0 + a1 * z1 + a2 * z2
    bd = b0 * z0 + b1 * z1 + b2 * z2
    cd = c0 * z0 + c1 * z1 + c2 * z2
    return dict(ok=ok, w0=(a0, b0, c0), w1=(a1, b1, c1), w2=(a2, b2, c2),
                dep=(ad, bd, cd))


def _split3_bf16(a):
    h = a.astype(bf16)
    r1 = a - h.astype(np.float64)
    m = r1.astype(bf16)
    l = (r1 - m.astype(np.float64)).astype(bf16)
    return h, m, l


def _bank_rows(a, b, c, sx, sy, mask_bad):
    """9 bf16 coefficient rows for one bank (recentered at sx, sy)."""
    cc = c + a * sx + b * sy
    a = np.where(mask_bad, 0.0, a)
    b = np.where(mask_bad, 0.0, b)
    cc = np.where(mask_bad, BADPOS, cc)
    ah, am, al = _split3_bf16(a)
    bh, bm, bl = _split3_bf16(b)
    ch, cm, cl = _split3_bf16(cc)
    return np.stack([ah, am, al, bh, bm, bl, ch, cm, cl], 0)   # [9, F] bf16


# ----------------------------------------------------------------------------
# Bass program
# ----------------------------------------------------------------------------

def _build_program(nch, fpad):
    import concourse.bacc as bacc
    import concourse.tile as tile
    import concourse.bass as bass
    from concourse import mybir
    from contextlib import ExitStack

    dt = mybir.dt
    op = mybir.AluOpType
    nc = bacc.Bacc("TRN2", target_bir_lowering=False, debug=False,
                   num_devices=NCORES)

    pixlhs = nc.dram_tensor("pixlhs", [128, nch * 128], dt.bfloat16, kind="ExternalInput")
    coefs = nc.dram_tensor("coefs", [128, fpad], dt.bfloat16, kind="ExternalInput")
    maxout = nc.dram_tensor("maxout", [128, nch], dt.float32, kind="ExternalOutput")
    idxout = nc.dram_tensor("idxout", [128, nch], dt.float32, kind="ExternalOutput")
    iotain = nc.dram_tensor("iotain", [128, fpad], dt.float32, kind="ExternalInput")

    assert fpad % BLK == 0
    nmm = fpad // BLK          # matmuls per bank (each writes a BLK-wide slice)

    with tile.TileContext(nc) as tc, ExitStack() as ctx:
        const = ctx.enter_context(tc.tile_pool(name="const", bufs=1))
        psum = ctx.enter_context(tc.tile_pool(name="psum", bufs=1, space="PSUM"))
        work = ctx.enter_context(tc.tile_pool(name="work", bufs=3))
        scorep = ctx.enter_context(tc.tile_pool(name="scorep", bufs=2))
        accp = ctx.enter_context(tc.tile_pool(name="accp", bufs=1))

        coef_sb = const.tile([128, fpad], dt.bfloat16)
        nc.sync.dma_start(out=coef_sb[:], in_=coefs[:])
        pix_sb = const.tile([128, nch * 128], dt.bfloat16)
        nc.sync.dma_start(out=pix_sb[:], in_=pixlhs[:])

        iota_f = const.tile([128, fpad], dt.float32)
        nc.sync.dma_start(out=iota_f[:], in_=iotain[:])

        maxall = accp.tile([128, nch], dt.float32)
        idxacc = accp.tile([128, nch], dt.float32)

        for j in range(nch):
            score = scorep.tile([128, fpad], dt.float32, tag="score")
            scr2 = scorep.tile([128, fpad], dt.float32, tag="scr2")
            # full-width PSUM banks (4 x fpad f32 = all 8 hw banks, bufs=1);
            # each bank is released by its (early) consumer so the next
            # chunk's matmuls chase the combine chain.
            banks = []
            for g in range(4):
                pt = psum.tile([128, fpad], dt.float32, tag=f"bank{g}")
                for b in range(nmm):
                    nc.tensor.matmul(
                        pt[:, b * BLK:(b + 1) * BLK],
                        pix_sb[32 * g:32 * g + 9, j * 128:(j + 1) * 128],
                        coef_sb[32 * g:32 * g + 9, b * BLK:(b + 1) * BLK],
                        start=True, stop=True,
                        tile_position=(32 * g, 0),
                    )
                banks.append(pt)
            # only ACT (copy) and DVE can read PSUM; Pool has no legal
            # elementwise path through this walrus pipeline.  Banks 0,1 are
            # cast to bf16 by ACT (half-width copies to cut chain latency) so
            # the first DVE max runs at the 16-bit 2x rate; banks 0..2 only
            # matter via sign/threshold so bf16 is safe, and the final max
            # with bank3 returns the f32 depth bit-exactly for covered pixels.
            c0b = work.tile([128, fpad], dt.bfloat16, tag="c0b")
            c1b = work.tile([128, fpad], dt.bfloat16, tag="c1b")
            h = fpad // 2
            nc.scalar.copy(out=c0b[:, 0:h], in_=banks[0][:, 0:h])
            nc.scalar.copy(out=c0b[:, h:], in_=banks[0][:, h:])
            nc.scalar.copy(out=c1b[:, 0:h], in_=banks[1][:, 0:h])
            nc.scalar.copy(out=c1b[:, h:], in_=banks[1][:, h:])
            t1 = work.tile([128, fpad], dt.bfloat16, tag="t1")
            nc.vector.tensor_tensor(out=t1[:], in0=c0b[:], in1=c1b[:], op=op.max)
            t2 = work.tile([128, fpad], dt.float32, tag="t2")
            nc.vector.tensor_tensor(out=t2[:], in0=t1[:], in1=banks[2][:], op=op.max)
            nc.vector.tensor_tensor(out=score[:], in0=t2[:], in1=banks[3][:], op=op.max)
            nc.vector.tensor_reduce(out=maxall[:, j:j + 1], in_=score[:],
                                    axis=mybir.AxisListType.X, op=op.min)
            # winner index: sum over f of (nS <= M) * f
            nc.vector.scalar_tensor_tensor(
                out=scr2[:], in0=score[:], scalar=maxall[:, j:j + 1],
                in1=iota_f[:], op0=op.is_le, op1=op.mult,
                accum_out=idxacc[:, j:j + 1],
            )

        nc.sync.dma_start(out=maxout[:], in_=maxall[:])
        nc.sync.dma_start(out=idxout[:], in_=idxacc[:])

    nc.compile()
    return nc


def _get_program(nch, fpad):
    key = (nch, fpad)
    if key not in _PROGRAM_CACHE:
        _PROGRAM_CACHE[key] = _build_program(nch, fpad)
    return _PROGRAM_CACHE[key]


# ----------------------------------------------------------------------------
# Host orchestration
# ----------------------------------------------------------------------------

def prepare(vertices, faces, textures, K, R, t, dist_coeffs):
    """All host-side prep.  Returns (nch, fpad, in_maps, scatter_info)."""
    verts = _project_f32(np.asarray(vertices), np.asarray(K), np.asarray(R),
                         np.asarray(t), np.asarray(dist_coeffs))
    fv = _face_vertices_f32(verts, np.asarray(faces))
    F = fv.shape[0]
    co = _build_coeffs(fv)
    bad = ~co['ok']
    tex = np.asarray(textures)[0].astype(np.float64)      # [F,3,C]

    # global bbox (+1px margin) of all face vertices
    xmin, xmax = fv[:, :, 0].min(), fv[:, :, 0].max()
    ymin, ymax = fv[:, :, 1].min(), fv[:, :, 1].max()
    c_lo = max(0, int(np.floor(xmin - 0.5)) - 1)
    c_hi = min(IMAGE_SIZE - 1, int(np.ceil(xmax - 0.5)) + 1)
    r_lo = max(0, int(np.floor(ymin - 0.5)) - 1)
    r_hi = min(IMAGE_SIZE - 1, int(np.ceil(ymax - 0.5)) + 1)
    if c_hi < c_lo or r_hi < r_lo:
        return None                                        # nothing visible

    ncols = c_hi - c_lo + 1
    nrows = r_hi - r_lo + 1
    G = nrows * ncols
    nch = max(1, -(-G // (NCORES * 128)))
    S = nch * 128

    # centered pixel coords must be bf16-exact small integers
    sx = np.floor((c_lo + c_hi) / 2) + 0.5
    sy = np.floor((r_lo + r_hi) / 2) + 0.5
    assert max(c_hi - c_lo, r_hi - r_lo) / 2 + 2 <= 192, "bbox too large"

    fpad = max(BLK, -(-F // BLK) * BLK)
    pad = np.zeros(fpad - F, dtype=bool)
    badp = np.concatenate([bad, ~pad])                     # padded faces are bad

    def padded(a):
        return np.concatenate([a, np.zeros(fpad - F, dtype=np.float64)])

    # banks: B_g = dep - BIG*w_g (g=0..2), B_3 = dep
    dep = tuple(padded(v) for v in co['dep'])
    coefs = np.zeros((128, fpad), dtype=bf16)
    for g, name in enumerate(['w0', 'w1', 'w2']):
        w = tuple(padded(v) for v in co[name])
        a = dep[0] - BIG * w[0]
        b = dep[1] - BIG * w[1]
        c = dep[2] - BIG * w[2]
        coefs[32 * g:32 * g + 9, :] = _bank_rows(a, b, c, sx, sy, badp)
    coefs[96:96 + 9, :] = _bank_rows(dep[0], dep[1], dep[2], sx, sy, badp)

    # host-side color affine coefficients [F, 3ch, 3] f64 (recentered)
    colco = np.zeros((fpad, 3, 3), dtype=np.float64)
    for ch in range(3):
        t0, t1, t2 = tex[:, 0, ch], tex[:, 1, ch], tex[:, 2, ch]
        A = padded(co['w0'][0] * t0 + co['w1'][0] * t1 + co['w2'][0] * t2)
        B = padded(co['w0'][1] * t0 + co['w1'][1] * t1 + co['w2'][1] * t2)
        C = padded(co['w0'][2] * t0 + co['w1'][2] * t1 + co['w2'][2] * t2)
        colco[:, ch, 0] = A
        colco[:, ch, 1] = B
        colco[:, ch, 2] = C + A * sx + B * sy

    # pixel lists per core: global bbox pixel g -> core g%8, slot g//8
    # slot s <-> (partition p, chunk j): s = p*nch + j
    iota_np = np.broadcast_to(np.arange(fpad, dtype=np.float32), (128, fpad)).copy()
    in_maps = []
    rows_of = np.empty((NCORES, S), dtype=np.int32)
    cols_of = np.empty((NCORES, S), dtype=np.int32)
    real_of = np.empty((NCORES, S), dtype=bool)
    pxc_of = np.empty((NCORES, S), dtype=np.float64)
    pyc_of = np.empty((NCORES, S), dtype=np.float64)
    for k in range(NCORES):
        g = np.arange(S) * NCORES + k
        real = g < G
        gc = np.where(real, g, 0)
        rr = gc // ncols + r_lo
        cc = gc % ncols + c_lo
        rows_of[k] = rr; cols_of[k] = cc; real_of[k] = real
        pxc = (cc + 0.5) - sx                              # exact ints
        pyc = (rr + 0.5) - sy
        pxc_of[k] = pxc; pyc_of[k] = pyc
        # s = p*nch + j ; lhsT column index within chunk j is p
        s = np.arange(S)
        p = s // nch
        j = s % nch
        colidx = j * 128 + p
        pixlhs = np.zeros((128, S), dtype=bf16)
        pxb = pxc.astype(bf16); pyb = pyc.astype(bf16)
        assert np.all(pxb.astype(np.float64) == pxc)
        assert np.all(pyb.astype(np.float64) == pyc)
        onerow = np.ones(S, dtype=bf16)
        rows9 = [pxb, pxb, pxb, pyb, pyb, pyb, onerow, onerow, onerow]
        for gg in range(4):
            for r in range(9):
                pixlhs[32 * gg + r, colidx] = rows9[r]
        in_maps.append(dict(pixlhs=pixlhs, coefs=coefs, iotain=iota_np))

    scatter = dict(rows_of=rows_of, cols_of=cols_of, real_of=real_of,
                   pxc_of=pxc_of, pyc_of=pyc_of, colco=colco,
                   nch=nch, fpad=fpad)
    return nch, fpad, in_maps, scatter


def assemble(results, scatter):
    out = np.zeros((1, 3, IMAGE_SIZE, IMAGE_SIZE), dtype=np.float32)
    nch = scatter['nch']
    fpad = scatter['fpad']
    colco = scatter['colco']
    for k in range(NCORES):
        mx = np.asarray(results[k]['maxout'], dtype=np.float64)  # [128, nch]
        ix = np.asarray(results[k]['idxout'], dtype=np.float64)
        # slot s = p*nch + j  ->  flatten [p, j] C-order gives slot order
        mxs = mx.reshape(-1)
        ixs = np.rint(ix.reshape(-1)).astype(np.int64)
        valid = (mxs < THRESH_POS) & (ixs >= 0) & (ixs < fpad)
        sel = scatter['real_of'][k] & valid
        if not np.any(sel):
            continue
        idx = ixs[sel]
        px = scatter['pxc_of'][k][sel]
        py = scatter['pyc_of'][k][sel]
        A = colco[idx]                                     # [n, 3ch, 3]
        col = A[:, :, 0] * px[:, None] + A[:, :, 1] * py[:, None] + A[:, :, 2]
        rr = scatter['rows_of'][k][sel]
        cc = scatter['cols_of'][k][sel]
        out[0, :, rr, cc] = col.astype(np.float32)
    return out


def kernel(**inputs):
    from concourse.bass_utils import run_bass_kernel_spmd

    prep = prepare(**inputs)
    if prep is None:
        return np.zeros((1, 3, IMAGE_SIZE, IMAGE_SIZE), dtype=np.float32)
    nch, fpad, in_maps, scatter = prep
    nc = _get_program(nch, fpad)
    res = run_bass_kernel_spmd(nc, in_maps, core_ids=list(range(NCORES)))
    return assemble(res.results, scatter)


if __name__ == "__main__":
    pass
